# revision 5
# baseline (speedup 1.0000x reference)
"""MoE (8 experts, top-2 routing) kernel for Trainium2 — 8 NeuronCores.

Sharding: full 8-way hidden-split.  Every core runs ALL 8 experts over
their exact routed token groups, but only a 512-wide slice of the
hidden dimension (H/8); core c takes hidden columns [512c, 512c+512).
Per-core work is therefore exactly sum(cnt_e) = 8192 token-equivalents
regardless of routing imbalance — the absolute row floor (524288
moving rows) with zero capacity padding, and the program is perfectly
SPMD (capacities = actual counts, identical on every core; only the
DRAM weight slices differ per core).  Weight traffic per core is
unchanged vs a 2-expert split (each core still streams one expert's
worth of W1+W2 bytes); x and partial-y traffic grow to ~17 MB each,
well under the DMA roofline with loads and stores spread over
separate HWDGE queues (sync: W1/x; gpsimd: b1/W2; stores alternate
scalar/vector).

The small gate runs host-side; the host gathers tokens per expert
(exact counts, no padding) and sums the 8 per-core hidden-slice
partials, applying combine weights and b2 there (exact for any b2).

Device kernel (per core), all matmuls bf16 (fp32 PSUM accumulation),
experts processed in descending-count order so the tail store is the
smallest:
  for each expert slot s:
    mm1 (token-tile-major, tiles paired per W1 chunk so each weight
        load covers two matmuls): hT[mi][:, tile] = relu(W1_mi^T x^T
        + b1) -> bf16   (mi over MH=4 hidden chunks)
    mm2 (128-token PSUM tiles): py[t] = sum_mi hT[mi][:, t]^T @ W2_mi
        — the whole H/8 contraction accumulates in one PSUM residency;
        each hts weight load serves both 512-wide halves of D.
    flush: PSUM->SBUF copy on the scalar engine; stores alternate
        between the scalar and sync HWDGE queues (the final store
        splits across both to shorten the tail).
Queue plan (only SP/ACT/gpsimd can initiate DMAs): SP = x loads +
half the stores; gpsimd = all weight loads (W1/b1/W2); ACT = the
other half of the stores.  Load emission is software-pipelined with
the expert loop (slot s+2's loads are emitted at the top of expert
s) so store kicks never queue behind stalled far-future load kicks
on the same in-order queue; pool buffer counts are sized so a
depth-2 prefetch never blocks a queue.  Weights stream through SBUF
exactly once; W1 of the running expert is fully resident (8 KB/
partition bf16).  DMA layouts are packed so every load is a
contiguous multi-KB burst per partition (W1 chunk-major
[P, MH, DK, P]; x tile-blocked [P, DK*tsz])."""

import numpy as np

P = 128
D = 1024
H = 4096
E = 8
HS = H // E       # per-core hidden slice (8-way split)
TOPK = 2
DK = D // P       # 8  contraction chunks for mm1
MH = HS // P      # 4  hidden chunks per core per expert


def _mm1_tiles(C, head=None):
    """Split C tokens into equal matmul moving-dim chunks (<=512 for one
    PSUM bank; chunks of ~280+ keep the bf16 matmul ahead of its 107 ns
    LDWEIGHTS so the PE is row-paced, not weight-load-paced).  An
    optional smaller head tile shrinks the DMA bytes the very first
    PSUM group waits on at kernel launch."""
    tiles = []
    if head and C - head > 512 and -(-(C - head) // 512) < -(-C // 512):
        tiles.append(head)
        C -= head
    n = max(1, -(-C // 512))
    base, r = divmod(C, n)
    return tiles + [base + (1 if i < r else 0) for i in range(n)]


def _build_program(counts):
    import concourse.mybir as mybir
    import concourse.tile as tile
    from concourse import bacc

    f32 = mybir.dt.float32
    bf16 = mybir.dt.bfloat16
    Relu = mybir.ActivationFunctionType.Relu
    Copy = mybir.ActivationFunctionType.Copy
    tts = [_mm1_tiles(C) for C in counts]
    Cmax = max(counts)

    nc = bacc.Bacc(
        "TRN2",
        target_bir_lowering=False,
        debug=False,
        enable_asserts=False,
        num_devices=E,
    )
    x_d, w1_d, w2_d, b1_d, y_d = [], [], [], [], []
    for s, C in enumerate(counts):
        # x tile-blocked: [P, sum_t DK*tsz_t] with per-tile [DK, tsz]
        # blocks so each tile loads as one contiguous burst per partition
        x_d.append(
            nc.dram_tensor(f"x{s}", [P, DK * C], bf16, kind="ExternalInput").ap()
        )
        # W1 chunk-major: [P, MH, DK, P] so each per-chunk DMA moves a
        # contiguous 2 KB per partition (full-rate DMA bursts)
        w1_d.append(
            nc.dram_tensor(f"w1_{s}", [P, MH, DK, P], bf16,
                           kind="ExternalInput").ap()
        )
        w2_d.append(
            nc.dram_tensor(f"w2_{s}", [HS, D], bf16, kind="ExternalInput").ap()
        )
        b1_d.append(
            nc.dram_tensor(f"b1_{s}", [P, MH], f32, kind="ExternalInput").ap()
        )
        # bf16 partials halve the store traffic (error budget: ~1e-3 of
        # a 2e-2 tolerance); the host sums the 8 hidden-slice partials
        y_d.append(
            nc.dram_tensor(f"y{s}", [C, D], bf16, kind="ExternalOutput").ap()
        )

    with tile.TileContext(nc) as tc:
        with (
            tc.tile_pool(name="const", bufs=1) as const,
            tc.tile_pool(name="xp", bufs=9) as xp,
            tc.tile_pool(name="w1p", bufs=3) as w1p,
            tc.tile_pool(name="w2p", bufs=3 * MH) as w2p,
            tc.tile_pool(name="htp", bufs=2 * MH) as htp,
            tc.tile_pool(name="ysp", bufs=3) as ysp,
            tc.tile_pool(name="php", bufs=4, space="PSUM") as php,
            tc.tile_pool(name="pyp", bufs=2, space="PSUM") as pyp,
        ):
            # ---- PE warm-up on a zeroed tile: ramps the DVFS while the
            # first operands stream in.  Emitted FIRST so the memset and
            # spin matmuls are the head of their engines' instruction
            # streams (a memset queued behind throttled DMA kicks would
            # deadlock the warmup).
            warm = const.tile([P, P], bf16)
            nc.any.memset(warm[:], 0.0)
            pw = php.tile([P, 512], f32, tag="ph", name="pw")
            # ~5.4us of spin, sized to the median first-operand arrival:
            # an idle gap between warmup and real work resets the DVFS
            # ramp and the first real matmuls run at the cold clock again
            for _ in range(52):
                nc.tensor.matmul(
                    pw[:, :P], warm[:], warm[:], start=True, stop=True
                )

            # ---- DMA load emission (per-queue in-order) ---------------
            # gpsimd queue: W1 (slot 0 per-chunk so the tensor engine
            # chases the stream through mm1 tile 0) + b1 + W2.
            # SP queue: x tiles.  Pools are sized so a depth-2 prefetch
            # (slots s..s+2 alive) never blocks either queue.
            w1t, xts, w2t, b1t = [], [], [], []

            def emit_loads(s):
                C = counts[s]
                w1 = w1p.tile([P, MH, DK, P], bf16, tag="w1", name=f"w1t_{s}")
                xtiles = []
                if s == 0:
                    nc.gpsimd.dma_start(w1[:, 0], w1_d[s][:, 0])
                    t0 = xp.tile([P, DK, tts[s][0]], bf16, tag="x",
                                 name=f"x_{s}_0")
                    nc.sync.dma_start(t0[:], x_d[s][:, 0:DK * tts[s][0]])
                    xtiles.append(t0)
                    for mi in range(1, MH):
                        nc.gpsimd.dma_start(w1[:, mi], w1_d[s][:, mi])
                    off = tts[s][0]
                else:
                    nc.gpsimd.dma_start(w1[:], w1_d[s][:])
                    off = 0
                for ti, tsz in enumerate(tts[s]):
                    if s == 0 and ti == 0:
                        continue
                    t = xp.tile([P, DK, tsz], bf16, tag="x", name=f"x_{s}_{ti}")
                    nc.sync.dma_start(t[:], x_d[s][:, DK * off:DK * (off + tsz)])
                    xtiles.append(t)
                    off += tsz
                w1t.append(w1)
                xts.append(xtiles)
                b1 = const.tile([P, MH], f32, name=f"b1t_{s}")
                nc.gpsimd.dma_start(b1[:], b1_d[s][:])
                b1t.append(b1)
                w2s = []
                for mi in range(MH):
                    t = w2p.tile([P, D], bf16, tag="w2", name=f"w2t_{s}_{mi}")
                    nc.gpsimd.dma_start(t[:], w2_d[s][mi * P:(mi + 1) * P, :])
                    w2s.append(t)
                w2t.append(w2s)

            emit_loads(0)
            emit_loads(1)

            # ---- per-expert compute -----------------------------------
            for s, C in enumerate(counts):
                if s + 2 < E:
                    emit_loads(s + 2)
                w1s, x_tiles, w2s, b1s = w1t[s], xts[s], w2t[s], b1t[s]
                hts = [
                    htp.tile([P, Cmax], bf16, tag="ht", name=f"ht_{s}_{mi}")
                    for mi in range(MH)
                ]
                # mm1: token tiles processed in pairs per weight chunk —
                # each W1 LDWEIGHTS serves two matmuls (into two PSUM
                # banks), keeping the weight load fully hidden.
                offs = []
                off = 0
                for tsz in tts[s]:
                    offs.append(off)
                    off += tsz
                start = len(tts[s]) % 2       # odd count: tile 0 solo first
                groups = [(0,)] * start + [
                    tuple(range(i, i + 2))
                    for i in range(start, len(tts[s]), 2)
                ]
                for grp in groups:
                    for mi in range(MH):
                        phs = [
                            php.tile([P, 512], f32, tag="ph",
                                     name=f"ph_{s}_{grp[0]}_{mi}_{g}")
                            for g in range(len(grp))
                        ]
                        for dk in range(DK):
                            for g, ti in enumerate(grp):
                                nc.tensor.matmul(
                                    phs[g][:, :tts[s][ti]],
                                    w1s[:, mi, dk, :],
                                    x_tiles[ti][:, dk, :],
                                    start=(dk == 0),
                                    stop=(dk == DK - 1),
                                )
                        for g, ti in enumerate(grp):
                            nc.scalar.activation(
                                hts[mi][:, offs[ti]:offs[ti] + tts[s][ti]],
                                phs[g][:, :tts[s][ti]], Relu,
                                bias=b1s[:, mi:mi + 1],
                            )
                # mm2: whole H/8 contraction in one PSUM residency per
                # 128-token tile; each hts weight load serves 2 matmuls
                # (h2 halves) so LDWEIGHTS stays hidden.  Plain copy
                # flush — combine weights are applied host-side.
                T = -(-C // P)
                for t in range(T):
                    np_ = min(P, C - t * P)
                    py = pyp.tile([P, D], f32, tag="py")
                    for mi in range(MH):
                        for h2 in range(2):
                            nc.tensor.matmul(
                                py[:np_, h2 * 512:(h2 + 1) * 512],
                                hts[mi][:, t * P:t * P + np_],
                                w2s[mi][:, h2 * 512:(h2 + 1) * 512],
                                start=(mi == 0),
                                stop=(mi == MH - 1),
                            )
                    ys = ysp.tile([P, D], bf16, tag="ys")
                    if s == E - 1 and t == T - 1:
                        # final tile: flush the two D-halves as separate
                        # scalar-engine copies and split the store across
                        # both store queues to shorten the drain tail
                        nc.scalar.activation(
                            ys[:np_, 0:512], py[:np_, 0:512], Copy
                        )
                        nc.sync.dma_start(
                            y_d[s][t * P:t * P + np_, 0:512], ys[:np_, 0:512]
                        )
                        nc.scalar.activation(
                            ys[:np_, 512:1024], py[:np_, 512:1024], Copy
                        )
                        nc.scalar.dma_start(
                            y_d[s][t * P:t * P + np_, 512:1024],
                            ys[:np_, 512:1024],
                        )
                    else:
                        nc.scalar.activation(ys[:np_], py[:np_], Copy)
                        # stores alternate between two HWDGE queues so
                        # the store stream never backlogs into the tail
                        eng = nc.scalar if t % 2 == 0 else nc.sync
                        eng.dma_start(y_d[s][t * P:t * P + np_, :], ys[:np_])
    nc.compile()
    return nc


def _route(x, Wg, bg):
    """Host gate: softmax over experts + stable top-2 (mirrors
    jax.lax.top_k tie-breaking: lowest index first)."""
    logits = x @ Wg + bg
    mx = logits.max(axis=1, keepdims=True)
    ex = np.exp(logits - mx)
    gate = ex / ex.sum(axis=1, keepdims=True)
    top2 = np.argsort(-gate, axis=1, kind="stable")[:, :TOPK]
    return gate, top2


def _pack_x(x, idx, C, bf16, head=None):
    """Gathered tokens -> [P, DK*C] bf16, tile-blocked: per mm1 tile a
    contiguous [DK, tsz] block per partition (single-burst DMA loads).
    `head` must match the device program's head-tile size."""
    xe = np.zeros((C, D), np.float32)
    xe[: len(idx)] = x[idx]
    blocks = []
    off = 0
    for tsz in _mm1_tiles(C, head=head):
        blk = xe[off:off + tsz].T.reshape(DK, P, tsz).transpose(1, 0, 2)
        blocks.append(blk.reshape(P, DK * tsz))
        off += tsz
    return np.ascontiguousarray(np.concatenate(blocks, axis=1)).astype(bf16)


def kernel(x, Wg, bg, W1, b1, W2, b2):
    import ml_dtypes
    from concourse.bass_utils import run_bass_kernel_spmd

    bf16 = ml_dtypes.bfloat16
    x = np.asarray(x, np.float32)
    Wg = np.asarray(Wg, np.float32)
    bg = np.asarray(bg, np.float32)
    W1 = np.asarray(W1, np.float32)
    b1 = np.asarray(b1, np.float32)
    W2 = np.asarray(W2, np.float32)
    b2 = np.asarray(b2, np.float32)
    Ttok = x.shape[0]

    gate, top2 = _route(x, Wg, bg)
    expert_idx = [
        np.nonzero((top2 == e).any(axis=1))[0] for e in range(E)
    ]
    cnts = np.array([len(s) for s in expert_idx])
    # slots in descending-count order: the last (smallest) expert leaves
    # the shortest store tail
    order = [int(e) for e in np.argsort(-cnts, kind="stable")]
    counts = tuple(max(P, int(cnts[e])) for e in order)

    nc = _build_program(counts)

    xs_packed = [
        _pack_x(x, expert_idx[e], counts[s], bf16)
        for s, e in enumerate(order)
    ]
    in_maps = []
    for c in range(E):
        hs = slice(c * HS, (c + 1) * HS)
        m = {}
        for s, e in enumerate(order):
            m[f"x{s}"] = xs_packed[s]
            m[f"w1_{s}"] = np.ascontiguousarray(
                W1[e][:, hs].reshape(DK, P, MH, P).transpose(1, 2, 0, 3)
            ).astype(bf16)
            m[f"w2_{s}"] = np.ascontiguousarray(W2[e][hs, :]).astype(bf16)
            m[f"b1_{s}"] = np.ascontiguousarray(b1[e][hs].reshape(MH, P).T)
        in_maps.append(m)

    def run_and_combine():
        results = run_bass_kernel_spmd(
            nc, in_maps, core_ids=list(range(E))
        ).results
        out = np.zeros((Ttok, D), np.float32)
        for s, e in enumerate(order):
            ia = expert_idx[e]
            acc = np.zeros((len(ia), D), np.float32)
            for c in range(E):
                acc += np.asarray(results[c][f"y{s}"][: len(ia)], np.float32)
            out[ia] += gate[ia, e][:, None] * acc
        # b2 contribution, folded on the host (exact for any b2)
        mask = np.zeros((Ttok, E), np.float32)
        np.put_along_axis(mask, top2, 1.0, axis=1)
        out += (gate * mask) @ b2
        return out

    def sample_ok(out):
        # Fail-open integrity check: recompute a few tokens on the host
        # and compare.  The device occasionally (rarely) returns garbage
        # after a transient fault; bf16 error is ~6e-3 abs, garbage is
        # O(1), so a 0.05 threshold separates them cleanly.
        try:
            toks = np.arange(0, Ttok, max(1, Ttok // 8))[:8]
            ref = np.zeros((len(toks), D), np.float32)
            for j, t in enumerate(toks):
                acc = np.zeros(D, np.float32)
                for e in top2[t]:
                    h = np.maximum(x[t] @ W1[e] + b1[e], 0.0)
                    acc = acc + gate[t, e] * (h @ W2[e] + b2[e])
                ref[j] = acc
            return float(np.abs(out[toks] - ref).max()) < 0.05
        except Exception:
            return True  # never let the checker break a good result

    out = run_and_combine()
    if not sample_ok(out):
        try:
            out2 = run_and_combine()
            if sample_ok(out2):
                out = out2
        except Exception:
            pass  # keep the first result if the retry itself fails
    return out


# revision 6
# speedup vs baseline: 1.2574x; 1.2574x over previous
"""MoE (8 experts, top-2 routing) kernel for Trainium2 — 8 NeuronCores.

Sharding: 4-way hidden-split over two expert quads.  Experts are split
into two quads of 4 by alternating descending-count ranks; quad q is
served by cores 4q..4q+3, core 4q+j taking hidden columns
[1024j, 1024j+1024) of all 4 experts in its quad.  Slot capacities are
the elementwise max of the two quads' sorted counts — the provable
minimum for a 2-group SPMD schedule (sum 4136 vs the 4096 ideal, ~1%
padding) — so per-core matmul rows sit ~1% off the absolute floor
while the program stays identical on every core.  An 8-way split has
~2% fewer rows but needs ~2x the x/partial-y DMA in half-length expert
phases, which oversubscribes the 16 DMA engines (~470 GB/s transient
demand vs ~336 GB/s capacity), starves the PE and anneals DVFS; 4-way
keeps sustained demand near 150 GB/s.

The small gate runs host-side; the host gathers tokens per expert
(quad-shared packs) and sums the 4 per-core hidden-slice partials,
applying combine weights and b2 there (exact for any b2).

Device kernel (per core), all matmuls bf16 (fp32 PSUM accumulation),
slots in descending-count order so the tail store is the smallest:
  for each expert slot s:
    mm1 (token-tile-major, tiles paired per W1 chunk so each weight
        load covers two matmuls): hT[mi][:, tile] = relu(W1_mi^T x^T
        + b1) -> bf16   (mi over MH=8 hidden chunks)
    mm2 (128-token PSUM tiles): py[t] = sum_mi hT[mi][:, t]^T @ W2_mi
        — the whole H/4 contraction accumulates in one PSUM residency;
        each hts weight load serves both 512-wide halves of D.
    flush: PSUM->SBUF copy on the scalar engine; stores alternate
        between the scalar and sync HWDGE queues (the final store
        splits across both to shorten the drain tail).
Queue plan (only SP/ACT/gpsimd can initiate DMAs): SP = x loads +
half the stores; gpsimd = all weight loads (W1/b1/W2); ACT = the
other half of the stores.  All loads are emitted up front in
first-use order; pool buffer rotation throttles each in-order queue
to ~1.5 slots of prefetch, and store kicks on SP only ever sit
behind x kicks that unblock strictly earlier than the store's data.
Weights stream through SBUF exactly once; W1 of the running expert
is fully resident (16 KB/partition bf16).  DMA layouts are packed so
every load is a contiguous multi-KB burst per partition (W1
chunk-major [P, MH, DK, P]; x tile-blocked [P, DK*tsz])."""

import numpy as np

P = 128
D = 1024
H = 4096
E = 8
NQ = 2            # expert quads
QE = E // NQ      # experts per quad (= slots per core)
HS = H // 4       # per-core hidden slice (4-way split within a quad)
TOPK = 2
DK = D // P       # 8  contraction chunks for mm1
MH = HS // P      # 8  hidden chunks per core per expert


def _mm1_tiles(C, head=None):
    """Split C tokens into equal matmul moving-dim chunks (<=512 for one
    PSUM bank; chunks of ~280+ keep the bf16 matmul ahead of its 107 ns
    LDWEIGHTS so the PE is row-paced, not weight-load-paced).  An
    optional smaller head tile shrinks the DMA bytes the very first
    PSUM group waits on at kernel launch."""
    tiles = []
    if head and C - head > 512 and -(-(C - head) // 512) < -(-C // 512):
        tiles.append(head)
        C -= head
    n = max(1, -(-C // 512))
    base, r = divmod(C, n)
    return tiles + [base + (1 if i < r else 0) for i in range(n)]


def _build_program(counts):
    import concourse.mybir as mybir
    import concourse.tile as tile
    from concourse import bacc

    f32 = mybir.dt.float32
    bf16 = mybir.dt.bfloat16
    Relu = mybir.ActivationFunctionType.Relu
    Copy = mybir.ActivationFunctionType.Copy
    tts = [_mm1_tiles(C) for C in counts]
    Cmax = max(counts)

    nc = bacc.Bacc(
        "TRN2",
        target_bir_lowering=False,
        debug=False,
        enable_asserts=False,
        num_devices=E,
    )
    x_d, w1_d, w2_d, y_d = [], [], [], []
    for s, C in enumerate(counts):
        # x tile-blocked: [P, sum_t DK*tsz_t] with per-tile [DK, tsz]
        # blocks so each tile loads as one contiguous burst per partition
        x_d.append(
            nc.dram_tensor(f"x{s}", [P, DK * C], bf16, kind="ExternalInput").ap()
        )
        # W1 chunk-major: [P, MH, DK, P] so each per-chunk DMA moves a
        # contiguous 2 KB per partition (full-rate DMA bursts)
        w1_d.append(
            nc.dram_tensor(f"w1_{s}", [P, MH, DK, P], bf16,
                           kind="ExternalInput").ap()
        )
        w2_d.append(
            nc.dram_tensor(f"w2_{s}", [HS, D], bf16, kind="ExternalInput").ap()
        )
        # bf16 partials halve the store traffic (error budget: ~1e-3 of
        # a 2e-2 tolerance); the host sums the 4 hidden-slice partials
        y_d.append(
            nc.dram_tensor(f"y{s}", [C, D], bf16, kind="ExternalOutput").ap()
        )
    # all slots' b1 slices in one tensor -> one load, one semaphore
    b1_d = nc.dram_tensor("b1all", [P, QE * MH], f32, kind="ExternalInput").ap()

    with tile.TileContext(nc) as tc:
        with (
            tc.tile_pool(name="const", bufs=1) as const,
            tc.tile_pool(name="xp", bufs=9) as xp,
            tc.tile_pool(name="w1p", bufs=4) as w1p,
            tc.tile_pool(name="w2p", bufs=12) as w2p,
            tc.tile_pool(name="htp", bufs=2 * MH) as htp,
            tc.tile_pool(name="ysp", bufs=3) as ysp,
            tc.tile_pool(name="php", bufs=4, space="PSUM") as php,
            tc.tile_pool(name="pyp", bufs=2, space="PSUM") as pyp,
        ):
            # ---- PE warm-up on a zeroed tile: ramps the DVFS while the
            # first operands stream in.  Emitted FIRST so the memset and
            # spin matmuls head their engines' instruction streams (a
            # memset queued behind throttled DMA kicks would deadlock
            # the warmup).
            warm = const.tile([P, P], bf16)
            nc.any.memset(warm[:], 0.0)
            pw = php.tile([P, 512], f32, tag="ph", name="pw")
            # ~5.4us of spin, sized to the median first-operand arrival
            # (~13us): an idle gap between warmup and real work resets
            # the DVFS ramp and the first real matmuls run at the cold
            # clock again
            for _ in range(52):
                nc.tensor.matmul(
                    pw[:, :P], warm[:], warm[:], start=True, stop=True
                )

            # ---- DMA load emission (per-queue in-order, first-use
            # order).  gpsimd queue: W1 (slot 0 per-chunk so the tensor
            # engine chases the stream through mm1 tile 0) + b1 + W2.
            # SP queue: x tiles.
            w1t, xts, w2t = [], [], []
            b1all = const.tile([P, QE * MH], f32)
            for s, C in enumerate(counts):
                w1 = w1p.tile([P, MH, DK, P], bf16, tag="w1", name=f"w1t_{s}")
                xtiles = []
                if s == 0:
                    nc.gpsimd.dma_start(w1[:, 0], w1_d[s][:, 0])
                    t0 = xp.tile([P, DK, tts[s][0]], bf16, tag="x",
                                 name=f"x_{s}_0")
                    nc.sync.dma_start(t0[:], x_d[s][:, 0:DK * tts[s][0]])
                    xtiles.append(t0)
                    for mi in range(1, 4):
                        nc.gpsimd.dma_start(w1[:, mi], w1_d[s][:, mi])
                    nc.gpsimd.dma_start(b1all[:], b1_d[:])
                    for mi in range(4, MH):
                        nc.gpsimd.dma_start(w1[:, mi], w1_d[s][:, mi])
                    off = tts[s][0]
                else:
                    nc.gpsimd.dma_start(w1[:], w1_d[s][:])
                    off = 0
                for ti, tsz in enumerate(tts[s]):
                    if s == 0 and ti == 0:
                        continue
                    t = xp.tile([P, DK, tsz], bf16, tag="x", name=f"x_{s}_{ti}")
                    nc.sync.dma_start(t[:], x_d[s][:, DK * off:DK * (off + tsz)])
                    xtiles.append(t)
                    off += tsz
                w1t.append(w1)
                xts.append(xtiles)
                w2s = []
                for mi in range(MH):
                    t = w2p.tile([P, D], bf16, tag="w2", name=f"w2t_{s}_{mi}")
                    nc.gpsimd.dma_start(t[:], w2_d[s][mi * P:(mi + 1) * P, :])
                    w2s.append(t)
                w2t.append(w2s)

            # ---- per-expert compute -----------------------------------
            for s, C in enumerate(counts):
                w1s, x_tiles, w2s = w1t[s], xts[s], w2t[s]
                hts = [
                    htp.tile([P, Cmax], bf16, tag="ht", name=f"ht_{s}_{mi}")
                    for mi in range(MH)
                ]
                # mm1: token tiles processed in pairs per weight chunk —
                # each W1 LDWEIGHTS serves two matmuls (into two PSUM
                # banks), keeping the weight load fully hidden.
                offs = []
                off = 0
                for tsz in tts[s]:
                    offs.append(off)
                    off += tsz
                start = len(tts[s]) % 2       # odd count: tile 0 solo first
                groups = [(0,)] * start + [
                    tuple(range(i, i + 2))
                    for i in range(start, len(tts[s]), 2)
                ]
                for grp in groups:
                    for mi in range(MH):
                        phs = [
                            php.tile([P, 512], f32, tag="ph",
                                     name=f"ph_{s}_{grp[0]}_{mi}_{g}")
                            for g in range(len(grp))
                        ]
                        for dk in range(DK):
                            for g, ti in enumerate(grp):
                                nc.tensor.matmul(
                                    phs[g][:, :tts[s][ti]],
                                    w1s[:, mi, dk, :],
                                    x_tiles[ti][:, dk, :],
                                    start=(dk == 0),
                                    stop=(dk == DK - 1),
                                )
                        for g, ti in enumerate(grp):
                            nc.scalar.activation(
                                hts[mi][:, offs[ti]:offs[ti] + tts[s][ti]],
                                phs[g][:, :tts[s][ti]], Relu,
                                bias=b1all[:, s * MH + mi:s * MH + mi + 1],
                            )
                # mm2: whole H/4 contraction in one PSUM residency per
                # 128-token tile; each hts weight load serves 2 matmuls
                # (h2 halves) so LDWEIGHTS stays hidden.  Plain copy
                # flush — combine weights are applied host-side.
                T = -(-C // P)
                for t in range(T):
                    np_ = min(P, C - t * P)
                    py = pyp.tile([P, D], f32, tag="py")
                    for mi in range(MH):
                        for h2 in range(2):
                            nc.tensor.matmul(
                                py[:np_, h2 * 512:(h2 + 1) * 512],
                                hts[mi][:, t * P:t * P + np_],
                                w2s[mi][:, h2 * 512:(h2 + 1) * 512],
                                start=(mi == 0),
                                stop=(mi == MH - 1),
                            )
                    ys = ysp.tile([P, D], bf16, tag="ys")
                    if s == QE - 1 and t == T - 1:
                        # final tile: flush the two D-halves as separate
                        # scalar-engine copies and split the store across
                        # both store queues to shorten the drain tail
                        nc.scalar.activation(
                            ys[:np_, 0:512], py[:np_, 0:512], Copy
                        )
                        nc.sync.dma_start(
                            y_d[s][t * P:t * P + np_, 0:512], ys[:np_, 0:512]
                        )
                        nc.scalar.activation(
                            ys[:np_, 512:1024], py[:np_, 512:1024], Copy
                        )
                        nc.scalar.dma_start(
                            y_d[s][t * P:t * P + np_, 512:1024],
                            ys[:np_, 512:1024],
                        )
                    else:
                        nc.scalar.activation(ys[:np_], py[:np_], Copy)
                        # stores alternate between two HWDGE queues so
                        # the store stream never backlogs into the tail
                        eng = nc.scalar if t % 2 == 0 else nc.sync
                        eng.dma_start(y_d[s][t * P:t * P + np_, :], ys[:np_])
    nc.compile()
    return nc


def _route(x, Wg, bg):
    """Host gate: softmax over experts + stable top-2 (mirrors
    jax.lax.top_k tie-breaking: lowest index first)."""
    logits = x @ Wg + bg
    mx = logits.max(axis=1, keepdims=True)
    ex = np.exp(logits - mx)
    gate = ex / ex.sum(axis=1, keepdims=True)
    top2 = np.argsort(-gate, axis=1, kind="stable")[:, :TOPK]
    return gate, top2


def _pack_x(x, idx, C, bf16, head=None):
    """Gathered tokens -> [P, DK*C] bf16, tile-blocked: per mm1 tile a
    contiguous [DK, tsz] block per partition (single-burst DMA loads).
    `head` must match the device program's head-tile size."""
    xe = np.zeros((C, D), np.float32)
    xe[: len(idx)] = x[idx]
    blocks = []
    off = 0
    for tsz in _mm1_tiles(C, head=head):
        blk = xe[off:off + tsz].T.reshape(DK, P, tsz).transpose(1, 0, 2)
        blocks.append(blk.reshape(P, DK * tsz))
        off += tsz
    return np.ascontiguousarray(np.concatenate(blocks, axis=1)).astype(bf16)


def kernel(x, Wg, bg, W1, b1, W2, b2):
    import ml_dtypes
    from concourse.bass_utils import run_bass_kernel_spmd

    bf16 = ml_dtypes.bfloat16
    x = np.asarray(x, np.float32)
    Wg = np.asarray(Wg, np.float32)
    bg = np.asarray(bg, np.float32)
    W1 = np.asarray(W1, np.float32)
    b1 = np.asarray(b1, np.float32)
    W2 = np.asarray(W2, np.float32)
    b2 = np.asarray(b2, np.float32)
    Ttok = x.shape[0]

    gate, top2 = _route(x, Wg, bg)
    expert_idx = [
        np.nonzero((top2 == e).any(axis=1))[0] for e in range(E)
    ]
    cnts = np.array([len(s) for s in expert_idx])
    # quads by alternating descending-count ranks: slot capacities are
    # then the elementwise max = the odd ranks, the provable minimum
    order = [int(e) for e in np.argsort(-cnts, kind="stable")]
    quads = [order[0::2], order[1::2]]    # slot s: quad0 rank 2s, quad1 2s+1
    counts = tuple(max(P, int(cnts[order[2 * s]])) for s in range(QE))

    nc = _build_program(counts)

    # x packs are shared by the 4 cores of a quad
    xs_packed = [
        [_pack_x(x, expert_idx[e], counts[s], bf16) for s, e in enumerate(qs)]
        for qs in quads
    ]
    in_maps = []
    for c in range(E):
        q, j = divmod(c, 4)
        hs = slice(j * HS, (j + 1) * HS)
        m = {}
        b1s = []
        for s, e in enumerate(quads[q]):
            m[f"x{s}"] = xs_packed[q][s]
            m[f"w1_{s}"] = np.ascontiguousarray(
                W1[e][:, hs].reshape(DK, P, MH, P).transpose(1, 2, 0, 3)
            ).astype(bf16)
            m[f"w2_{s}"] = np.ascontiguousarray(W2[e][hs, :]).astype(bf16)
            b1s.append(b1[e][hs].reshape(MH, P).T)
        m["b1all"] = np.ascontiguousarray(np.concatenate(b1s, axis=1))
        in_maps.append(m)

    def run_and_combine():
        results = run_bass_kernel_spmd(
            nc, in_maps, core_ids=list(range(E))
        ).results
        out = np.zeros((Ttok, D), np.float32)
        for q in range(NQ):
            for s, e in enumerate(quads[q]):
                ia = expert_idx[e]
                acc = np.zeros((len(ia), D), np.float32)
                for j in range(4):
                    acc += np.asarray(
                        results[4 * q + j][f"y{s}"][: len(ia)], np.float32
                    )
                out[ia] += gate[ia, e][:, None] * acc
        # b2 contribution, folded on the host (exact for any b2)
        mask = np.zeros((Ttok, E), np.float32)
        np.put_along_axis(mask, top2, 1.0, axis=1)
        out += (gate * mask) @ b2
        return out

    def sample_ok(out):
        # Fail-open integrity check: recompute a few tokens on the host
        # and compare.  The device occasionally (rarely) returns garbage
        # after a transient fault; bf16 error is ~6e-3 abs, garbage is
        # O(1), so a 0.05 threshold separates them cleanly.
        try:
            toks = np.arange(0, Ttok, max(1, Ttok // 8))[:8]
            ref = np.zeros((len(toks), D), np.float32)
            for j, t in enumerate(toks):
                acc = np.zeros(D, np.float32)
                for e in top2[t]:
                    h = np.maximum(x[t] @ W1[e] + b1[e], 0.0)
                    acc = acc + gate[t, e] * (h @ W2[e] + b2[e])
                ref[j] = acc
            return float(np.abs(out[toks] - ref).max()) < 0.05
        except Exception:
            return True  # never let the checker break a good result

    out = run_and_combine()
    if not sample_ok(out):
        try:
            out2 = run_and_combine()
            if sample_ok(out2):
                out = out2
        except Exception:
            pass  # keep the first result if the retry itself fails
    return out
